# revision 2
# baseline (speedup 1.0000x reference)
"""Trainium2 Bass kernel for the masked-GCN autoencoder (MAE+contrastive encoder).

Reference computation (N=10000 cells, F=2000 genes, dense adj [N,N]):
    h  = x * mask
    h0 = relu(adj @ (h  @ We0) + be0)       # [N, 512]
    h1 = relu(adj @ (h0 @ We1) + be1)       # [N, 256]
    z  = adj @ (h1 @ Wz) + bz               # [N, 128]
    z  = z / max(||z||_row, 1e-12)
    d0 = relu(z  @ Wd0 + bd0)
    d1 = relu(d0 @ Wd1 + bd1)
    x_hat = (d1 @ Wd2 + bd2) * mask
    returns (z, x_hat, mask)

Distribution: rows (cells) sharded 1280/core across 8 cores (N padded to
10240).  Each core holds adj[rows_c,:].T (so the contraction dim lies on
SBUF partitions), computes t = h @ W locally, AllGathers t across cores,
then computes its row block of adj @ t.  Weights are replicated.  Matmul
operands travel as bf16 (PSUM accumulation is fp32); outputs are fp32.
"""

import numpy as np
import ml_dtypes

import concourse.bass as bass
import concourse.mybir as mybir
import concourse.tile as tile
from concourse import bacc
from concourse.bass_utils import run_bass_kernel_spmd
from concourse.masks import make_identity

F32 = mybir.dt.float32
BF16 = mybir.dt.bfloat16
NP_BF16 = ml_dtypes.bfloat16

NCORES = 8
N, F = 10000, 2000
H0, H1, Z = 512, 256, 128
NPAD = 10240          # padded cell count, 8 * 1280
NLOC = NPAD // NCORES  # 1280 rows per core
MT = NLOC // 128       # 10 m-tiles per core
KADJ = NPAD // 128     # 80 contraction chunks for adj matmuls
FPAD = 2048
KF = FPAD // 128       # 16 contraction chunks for x @ We0


def _chunks(total, step=512):
    out = []
    off = 0
    while off < total:
        sz = min(step, total - off)
        out.append((off, sz))
        off += sz
    return out


NLOC_CH = _chunks(NLOC)   # [(0,512),(512,512),(1024,256)]
F_CH = _chunks(F)         # [(0,512),(512,512),(1024,512),(1536,464)]


def build_program():
    nc = bacc.Bacc("TRN2", target_bir_lowering=False, debug=False,
                   num_devices=NCORES)

    # ---- external I/O ----
    xT = nc.dram_tensor("xT", [FPAD, NLOC], BF16, kind="ExternalInput")
    mT = nc.dram_tensor("mT", [FPAD, NLOC], BF16, kind="ExternalInput")
    adjT = nc.dram_tensor("adjT", [NPAD, NLOC], BF16, kind="ExternalInput")
    mask = nc.dram_tensor("mask", [NLOC, F], F32, kind="ExternalInput")

    We0 = nc.dram_tensor("We0", [FPAD, H0], BF16, kind="ExternalInput")
    We1 = nc.dram_tensor("We1", [H0, H1], BF16, kind="ExternalInput")
    Wz = nc.dram_tensor("Wz", [H1, Z], BF16, kind="ExternalInput")
    Wd0 = nc.dram_tensor("Wd0", [Z, H1], BF16, kind="ExternalInput")
    Wd1 = nc.dram_tensor("Wd1", [H1, H0], BF16, kind="ExternalInput")
    Wd2 = nc.dram_tensor("Wd2", [H0, F], BF16, kind="ExternalInput")
    # biases, host-reshaped to [128, H/128] f32
    be0 = nc.dram_tensor("be0", [128, H0 // 128], F32, kind="ExternalInput")
    be1 = nc.dram_tensor("be1", [128, H1 // 128], F32, kind="ExternalInput")
    bz = nc.dram_tensor("bz", [128, Z // 128], F32, kind="ExternalInput")
    bd0 = nc.dram_tensor("bd0", [128, H1 // 128], F32, kind="ExternalInput")
    bd1 = nc.dram_tensor("bd1", [128, H0 // 128], F32, kind="ExternalInput")
    bd2 = nc.dram_tensor("bd2", [1, F], BF16, kind="ExternalInput")

    z_out = nc.dram_tensor("z_out", [NLOC, Z], F32, kind="ExternalOutput")
    xh_out = nc.dram_tensor("xh_out", [NLOC, F], F32, kind="ExternalOutput")

    with tile.TileContext(nc) as tc:
        with (
            tc.tile_pool(name="const", bufs=1) as const,
            tc.tile_pool(name="acts", bufs=1) as acts,
            tc.tile_pool(name="adj_p", bufs=3) as adj_p,
            tc.tile_pool(name="t_p", bufs=6) as t_p,
            tc.tile_pool(name="io_p", bufs=4) as io_p,
            tc.tile_pool(name="small", bufs=6) as small,
            tc.tile_pool(name="ps", bufs=8, space="PSUM") as ps_p,
            tc.tile_pool(name="dram", bufs=1, space="DRAM") as dram,
        ):
            # ---- replicated constants in SBUF ----
            We0_sb = const.tile([128, KF, H0], BF16)
            nc.sync.dma_start(We0_sb[:], We0.rearrange("(k p) h -> p k h", p=128))
            We1_sb = const.tile([128, H0 // 128, H1], BF16)
            nc.sync.dma_start(We1_sb[:], We1.rearrange("(k p) h -> p k h", p=128))
            Wz_sb = const.tile([128, H1 // 128, Z], BF16)
            nc.sync.dma_start(Wz_sb[:], Wz.rearrange("(k p) h -> p k h", p=128))
            Wd0_sb = const.tile([128, H1], BF16)
            nc.sync.dma_start(Wd0_sb[:], Wd0[:, :])
            Wd1_sb = const.tile([128, H1 // 128, H0], BF16)
            nc.sync.dma_start(Wd1_sb[:], Wd1.rearrange("(k p) h -> p k h", p=128))
            Wd2_sb = const.tile([128, H0 // 128, F], BF16)
            nc.sync.dma_start(Wd2_sb[:], Wd2.rearrange("(k p) h -> p k h", p=128))
            be0_sb = const.tile([128, H0 // 128], F32)
            nc.sync.dma_start(be0_sb[:], be0[:, :])
            be1_sb = const.tile([128, H1 // 128], F32)
            nc.sync.dma_start(be1_sb[:], be1[:, :])
            bz_sb = const.tile([128, Z // 128], F32)
            nc.sync.dma_start(bz_sb[:], bz[:, :])
            bd0_sb = const.tile([128, H1 // 128], F32)
            nc.sync.dma_start(bd0_sb[:], bd0[:, :])
            bd1_sb = const.tile([128, H0 // 128], F32)
            nc.sync.dma_start(bd1_sb[:], bd1[:, :])
            bd2_sb = const.tile([1, F], BF16)
            nc.sync.dma_start(bd2_sb[:], bd2[:, :])
            ones_sb = const.tile([1, 128], BF16)
            nc.any.memset(ones_sb[:], 1.0)
            ident = const.tile([128, 128], F32)
            make_identity(nc, ident[:])

            # ---- persistent activations (bf16, transposed layouts) ----
            hT_sb = acts.tile([128, KF, NLOC], BF16)    # (x*mask).T
            h0T_sb = acts.tile([128, H0 // 128, NLOC], BF16)
            h1T_sb = acts.tile([128, H1 // 128, NLOC], BF16)
            zT_sb = acts.tile([128, NLOC], F32)
            zsT_sb = acts.tile([128, NLOC], BF16)       # normalized z, transposed
            u0T_sb = acts.tile([128, H1 // 128, NLOC], BF16)
            u1T_sb = acts.tile([128, H0 // 128, NLOC], BF16)

            # ---- DRAM bounce buffers for the AllGathers ----
            t0loc = dram.tile([NLOC, H0], BF16)
            t0full = dram.tile([NPAD, H0], BF16, addr_space="Shared")
            t1loc = dram.tile([NLOC, H1], BF16)
            t1full = dram.tile([NPAD, H1], BF16, addr_space="Shared")
            tzloc = dram.tile([NLOC, Z], BF16)
            tzfull = dram.tile([NPAD, Z], BF16, addr_space="Shared")

            rg = [list(range(NCORES))]

            # ======== phase E: hT = (x*mask).T ; t0 = h @ We0 (local rows) ====
            for kf in range(KF):
                xt_t = io_p.tile([128, NLOC], BF16, tag="xt")
                nc.sync.dma_start(xt_t[:], xT[kf * 128:(kf + 1) * 128, :])
                mt_t = io_p.tile([128, NLOC], BF16, tag="mt")
                nc.sync.dma_start(mt_t[:], mT[kf * 128:(kf + 1) * 128, :])
                nc.vector.tensor_mul(hT_sb[:, kf, :], xt_t[:], mt_t[:])

            for m in range(MT):
                ps = ps_p.tile([128, H0], F32, tag="ps")
                for kf in range(KF):
                    nc.tensor.matmul(
                        ps[:], hT_sb[:, kf, m * 128:(m + 1) * 128],
                        We0_sb[:, kf, :],
                        start=(kf == 0), stop=(kf == KF - 1))
                stage = io_p.tile([128, H0], BF16, tag="stage")
                nc.scalar.copy(stage[:], ps[:])
                nc.sync.dma_start(t0loc[m * 128:(m + 1) * 128, :], stage[:])

            nc.gpsimd.collective_compute(
                "AllGather", mybir.AluOpType.bypass, replica_groups=rg,
                ins=[t0loc[:].opt()], outs=[t0full[:].opt()])

            # ======== phase L1: h0 = relu(adj @ t0 + be0), transposed out ====
            # free-dim groups keep live PSUM banks <= 8
            for grp in ([(0, 512), (512, 512)], [(1024, 256)]):
                g0 = grp[0][0]
                gw = sum(sz for _, sz in grp)
                pss = {}
                for (off, sz) in grp:
                    for hp in range(H0 // 128):
                        pss[(off, hp)] = ps_p.tile([128, sz], F32, tag="ps", name=f"psb_{off}_{hp}")
                for k in range(KADJ):
                    a_t = adj_p.tile([128, gw], BF16, tag="adj")
                    nc.sync.dma_start(a_t[:], adjT[k * 128:(k + 1) * 128,
                                                   g0:g0 + gw])
                    t_t = t_p.tile([128, H0], BF16, tag="tt")
                    nc.sync.dma_start(t_t[:], t0full[k * 128:(k + 1) * 128, :])
                    for (off, sz) in grp:
                        for hp in range(H0 // 128):
                            nc.tensor.matmul(
                                pss[(off, hp)][:],
                                t_t[:, hp * 128:(hp + 1) * 128],
                                a_t[:, off - g0:off - g0 + sz],
                                start=(k == 0), stop=(k == KADJ - 1))
                for (off, sz) in grp:
                    for hp in range(H0 // 128):
                        nc.scalar.activation(
                            h0T_sb[:, hp, off:off + sz], pss[(off, hp)][:],
                            mybir.ActivationFunctionType.Relu,
                            bias=be0_sb[:, hp:hp + 1])

            # ======== phase P1: t1 = h0 @ We1 (local rows) ========
            for m in range(MT):
                ps = ps_p.tile([128, H1], F32, tag="ps")
                for hk in range(H0 // 128):
                    nc.tensor.matmul(
                        ps[:], h0T_sb[:, hk, m * 128:(m + 1) * 128],
                        We1_sb[:, hk, :],
                        start=(hk == 0), stop=(hk == H0 // 128 - 1))
                stage = io_p.tile([128, H1], BF16, tag="stage")
                nc.scalar.copy(stage[:], ps[:])
                nc.sync.dma_start(t1loc[m * 128:(m + 1) * 128, :], stage[:])

            nc.gpsimd.collective_compute(
                "AllGather", mybir.AluOpType.bypass, replica_groups=rg,
                ins=[t1loc[:].opt()], outs=[t1full[:].opt()])

            # ======== phase L2: h1 = relu(adj @ t1 + be1) ========
            pss = {}
            for (off, sz) in NLOC_CH:
                for hp in range(H1 // 128):
                    pss[(off, hp)] = ps_p.tile([128, sz], F32, tag="ps", name=f"psb_{off}_{hp}")
            for k in range(KADJ):
                a_t = adj_p.tile([128, NLOC], BF16, tag="adj")
                nc.sync.dma_start(a_t[:], adjT[k * 128:(k + 1) * 128, :])
                t_t = t_p.tile([128, H1], BF16, tag="tt")
                nc.sync.dma_start(t_t[:], t1full[k * 128:(k + 1) * 128, :])
                for (off, sz) in NLOC_CH:
                    for hp in range(H1 // 128):
                        nc.tensor.matmul(
                            pss[(off, hp)][:],
                            t_t[:, hp * 128:(hp + 1) * 128],
                            a_t[:, off:off + sz],
                            start=(k == 0), stop=(k == KADJ - 1))
            for (off, sz) in NLOC_CH:
                for hp in range(H1 // 128):
                    nc.scalar.activation(
                        h1T_sb[:, hp, off:off + sz], pss[(off, hp)][:],
                        mybir.ActivationFunctionType.Relu,
                        bias=be1_sb[:, hp:hp + 1])

            # ======== phase P2: tz = h1 @ Wz ========
            for m in range(MT):
                ps = ps_p.tile([128, Z], F32, tag="ps")
                for hk in range(H1 // 128):
                    nc.tensor.matmul(
                        ps[:], h1T_sb[:, hk, m * 128:(m + 1) * 128],
                        Wz_sb[:, hk, :],
                        start=(hk == 0), stop=(hk == H1 // 128 - 1))
                stage = io_p.tile([128, Z], BF16, tag="stage")
                nc.scalar.copy(stage[:], ps[:])
                nc.sync.dma_start(tzloc[m * 128:(m + 1) * 128, :], stage[:])

            nc.gpsimd.collective_compute(
                "AllGather", mybir.AluOpType.bypass, replica_groups=rg,
                ins=[tzloc[:].opt()], outs=[tzfull[:].opt()])

            # ======== phase L3: z = adj @ tz + bz (no relu) ========
            pss = {}
            for (off, sz) in NLOC_CH:
                pss[off] = ps_p.tile([128, sz], F32, tag="ps", name=f"psb_{off}")
            for k in range(KADJ):
                a_t = adj_p.tile([128, NLOC], BF16, tag="adj")
                nc.sync.dma_start(a_t[:], adjT[k * 128:(k + 1) * 128, :])
                t_t = t_p.tile([128, Z], BF16, tag="tt")
                nc.sync.dma_start(t_t[:], tzfull[k * 128:(k + 1) * 128, :])
                for (off, sz) in NLOC_CH:
                    nc.tensor.matmul(
                        pss[off][:], t_t[:], a_t[:, off:off + sz],
                        start=(k == 0), stop=(k == KADJ - 1))
            for (off, sz) in NLOC_CH:
                nc.scalar.activation(
                    zT_sb[:, off:off + sz], pss[off][:],
                    mybir.ActivationFunctionType.Identity,
                    bias=bz_sb[:, 0:1])

            # ======== phase Z: normalize rows of z, emit z_out and zsT ======
            for m in range(MT):
                ms = slice(m * 128, (m + 1) * 128)
                pst = ps_p.tile([128, 128], F32, tag="ps")
                nc.tensor.transpose(pst[:], zT_sb[:, ms], ident[:])
                # row sum of squares -> norm -> reciprocal
                sq = small.tile([128, 128], F32, tag="sq")
                nc.scalar.square(sq[:], pst[:])
                ssum = small.tile([128, 1], F32, tag="ssum")
                nc.vector.reduce_sum(ssum[:], sq[:], axis=mybir.AxisListType.X)
                nc.scalar.sqrt(ssum[:], ssum[:])
                nc.vector.tensor_scalar_max(ssum[:], ssum[:], 1e-12)
                rinv = small.tile([128, 1], F32, tag="rinv")
                nc.vector.reciprocal(rinv[:], ssum[:])
                zn = small.tile([128, 128], F32, tag="zn")
                nc.vector.tensor_scalar_mul(zn[:], pst[:], rinv[:])
                nc.sync.dma_start(z_out[ms, :], zn[:])
                # transpose normalized block back for the decoder
                pst2 = ps_p.tile([128, 128], F32, tag="ps")
                nc.tensor.transpose(pst2[:], zn[:], ident[:])
                nc.vector.tensor_copy(zsT_sb[:, ms], pst2[:])

            # ======== phase D: decoder ========
            # u0 = relu(z @ Wd0 + bd0), transposed: u0T = Wd0.T @ zsT
            for hp in range(H1 // 128):
                for (off, sz) in NLOC_CH:
                    ps = ps_p.tile([128, sz], F32, tag="ps")
                    nc.tensor.matmul(
                        ps[:], Wd0_sb[:, hp * 128:(hp + 1) * 128],
                        zsT_sb[:, off:off + sz], start=True, stop=True)
                    nc.scalar.activation(
                        u0T_sb[:, hp, off:off + sz], ps[:],
                        mybir.ActivationFunctionType.Relu,
                        bias=bd0_sb[:, hp:hp + 1])
            # u1 = relu(u0 @ Wd1 + bd1), transposed
            for hp in range(H0 // 128):
                for (off, sz) in NLOC_CH:
                    ps = ps_p.tile([128, sz], F32, tag="ps")
                    for kp in range(H1 // 128):
                        nc.tensor.matmul(
                            ps[:], Wd1_sb[:, kp, hp * 128:(hp + 1) * 128],
                            u0T_sb[:, kp, off:off + sz],
                            start=(kp == 0), stop=(kp == H1 // 128 - 1))
                    nc.scalar.activation(
                        u1T_sb[:, hp, off:off + sz], ps[:],
                        mybir.ActivationFunctionType.Relu,
                        bias=bd1_sb[:, hp:hp + 1])
            # x_hat = (u1 @ Wd2 + bd2) * mask, natural layout
            for m in range(MT):
                ms = slice(m * 128, (m + 1) * 128)
                for (off, sz) in F_CH:
                    ps = ps_p.tile([128, sz], F32, tag="ps")
                    for kp in range(H0 // 128):
                        nc.tensor.matmul(
                            ps[:], u1T_sb[:, kp, ms],
                            Wd2_sb[:, kp, off:off + sz],
                            start=(kp == 0), stop=False)
                    nc.tensor.matmul(
                        ps[:], ones_sb[:, :], bd2_sb[:, off:off + sz],
                        start=False, stop=True)
                    mk_t = io_p.tile([128, 512], F32, tag="mk")
                    nc.sync.dma_start(mk_t[:, :sz], mask[ms, off:off + sz])
                    xh_t = io_p.tile([128, 512], F32, tag="xh")
                    nc.vector.tensor_mul(xh_t[:, :sz], ps[:], mk_t[:, :sz])
                    nc.sync.dma_start(xh_out[ms, off:off + sz], xh_t[:, :sz])

    nc.compile()
    return nc


_PROGRAM_CACHE = {}


def _get_program():
    if "nc" not in _PROGRAM_CACHE:
        _PROGRAM_CACHE["nc"] = build_program()
    return _PROGRAM_CACHE["nc"]


def _bias_cols(b, h):
    # [H] f32 -> [128, H/128] with b_out[p, a] = b[a*128 + p]
    return np.ascontiguousarray(
        np.asarray(b, dtype=np.float32).reshape(h // 128, 128).T)


def shard_inputs(x, adj, mask, We0, be0, We1, be1, Wz, bz,
                 Wd0, bd0, Wd1, bd1, Wd2, bd2):
    x = np.asarray(x, dtype=np.float32)
    adj = np.asarray(adj, dtype=np.float32)
    mask = np.asarray(mask, dtype=np.float32)

    We0p = np.zeros((FPAD, H0), NP_BF16)
    We0p[:F] = np.asarray(We0, np.float32).astype(NP_BF16)
    common = {
        "We0": We0p,
        "We1": np.asarray(We1, np.float32).astype(NP_BF16),
        "Wz": np.asarray(Wz, np.float32).astype(NP_BF16),
        "Wd0": np.asarray(Wd0, np.float32).astype(NP_BF16),
        "Wd1": np.asarray(Wd1, np.float32).astype(NP_BF16),
        "Wd2": np.asarray(Wd2, np.float32).astype(NP_BF16),
        "be0": _bias_cols(be0, H0),
        "be1": _bias_cols(be1, H1),
        "bz": _bias_cols(bz, Z),
        "bd0": _bias_cols(bd0, H1),
        "bd1": _bias_cols(bd1, H0),
        "bd2": np.asarray(bd2, np.float32).astype(NP_BF16).reshape(1, F),
    }

    in_maps = []
    for c in range(NCORES):
        r0 = c * NLOC
        r1 = min((c + 1) * NLOC, N)
        nr = r1 - r0

        xT_c = np.zeros((FPAD, NLOC), NP_BF16)
        xT_c[:F, :nr] = x[r0:r1].T.astype(NP_BF16)
        mT_c = np.zeros((FPAD, NLOC), NP_BF16)
        mT_c[:F, :nr] = mask[r0:r1].T.astype(NP_BF16)
        adjT_c = np.zeros((NPAD, NLOC), NP_BF16)
        adjT_c[:N, :nr] = adj[r0:r1].T.astype(NP_BF16)
        mask_c = np.zeros((NLOC, F), np.float32)
        mask_c[:nr] = mask[r0:r1]

        in_maps.append({
            "xT": xT_c, "mT": mT_c, "adjT": adjT_c, "mask": mask_c,
            **common,
        })
    return in_maps


def run_sharded(in_maps, trace=False, **kwargs):
    nc = _get_program()
    return run_bass_kernel_spmd(nc, in_maps, core_ids=list(range(NCORES)),
                                trace=trace, **kwargs)


def assemble_outputs(results, mask):
    z = np.concatenate([results[c]["z_out"] for c in range(NCORES)],
                       axis=0)[:N]
    xh = np.concatenate([results[c]["xh_out"] for c in range(NCORES)],
                        axis=0)[:N]
    mask = np.asarray(mask, dtype=np.float32)
    return z, xh, mask


def kernel(x, adj, mask, We0, be0, We1, be1, Wz, bz,
           Wd0, bd0, Wd1, bd1, Wd2, bd2):
    in_maps = shard_inputs(x, adj, mask, We0, be0, We1, be1, Wz, bz,
                           Wd0, bd0, Wd1, bd1, Wd2, bd2)
    res = run_sharded(in_maps)
    return assemble_outputs(res.results, mask)


# revision 3
# speedup vs baseline: 1.1597x; 1.1597x over previous
"""Trainium2 Bass kernel for the masked-GCN autoencoder (MAE+contrastive encoder).

Reference computation (N=10000 cells, F=2000 genes, dense adj [N,N]):
    h  = x * mask
    h0 = relu(adj @ (h  @ We0) + be0)       # [N, 512]
    h1 = relu(adj @ (h0 @ We1) + be1)       # [N, 256]
    z  = adj @ (h1 @ Wz) + bz               # [N, 128]
    z  = z / max(||z||_row, 1e-12)
    d0 = relu(z  @ Wd0 + bd0)
    d1 = relu(d0 @ Wd1 + bd1)
    x_hat = (d1 @ Wd2 + bd2) * mask
    returns (z, x_hat, mask)

Distribution: rows (cells) sharded 1280/core across 8 cores (N padded to
10240).  Each core holds adj[rows_c,:].T (so the contraction dim lies on
SBUF partitions), computes t = h @ W locally, AllGathers t across cores,
then computes its row block of adj @ t.  Weights are replicated.  Matmul
operands travel as bf16 (PSUM accumulation is fp32); outputs are fp32.
"""

import numpy as np
import ml_dtypes

import concourse.bass as bass
import concourse.mybir as mybir
import concourse.tile as tile
from concourse import bacc
from concourse.bass_utils import run_bass_kernel_spmd
from concourse.masks import make_identity

F32 = mybir.dt.float32
BF16 = mybir.dt.bfloat16
NP_BF16 = ml_dtypes.bfloat16

NCORES = 8
N, F = 10000, 2000
H0, H1, Z = 512, 256, 128
NPAD = 10240          # padded cell count, 8 * 1280
NLOC = NPAD // NCORES  # 1280 rows per core
MT = NLOC // 128       # 10 m-tiles per core
KADJ = NPAD // 128     # 80 contraction chunks for adj matmuls
FPAD = 2048
KF = FPAD // 128       # 16 contraction chunks for x @ We0


def _chunks(total, step=512):
    out = []
    off = 0
    while off < total:
        sz = min(step, total - off)
        out.append((off, sz))
        off += sz
    return out


NLOC_CH = _chunks(NLOC)   # [(0,512),(512,512),(1024,256)]
F_CH = _chunks(F)         # [(0,512),(512,512),(1024,512),(1536,464)]


def build_program():
    nc = bacc.Bacc("TRN2", target_bir_lowering=False, debug=False,
                   num_devices=NCORES)

    # ---- external I/O ----
    xT = nc.dram_tensor("xT", [FPAD, NLOC], BF16, kind="ExternalInput")
    mT = nc.dram_tensor("mT", [FPAD, NLOC], BF16, kind="ExternalInput")
    adjT = nc.dram_tensor("adjT", [NPAD, NLOC], BF16, kind="ExternalInput")
    mask = nc.dram_tensor("mask", [NLOC, F], F32, kind="ExternalInput")

    We0 = nc.dram_tensor("We0", [FPAD, H0], BF16, kind="ExternalInput")
    We1 = nc.dram_tensor("We1", [H0, H1], BF16, kind="ExternalInput")
    Wz = nc.dram_tensor("Wz", [H1, Z], BF16, kind="ExternalInput")
    Wd0 = nc.dram_tensor("Wd0", [Z, H1], BF16, kind="ExternalInput")
    Wd1 = nc.dram_tensor("Wd1", [H1, H0], BF16, kind="ExternalInput")
    Wd2 = nc.dram_tensor("Wd2", [H0, F], BF16, kind="ExternalInput")
    # biases, host-reshaped to [128, H/128] f32
    be0 = nc.dram_tensor("be0", [128, H0 // 128], F32, kind="ExternalInput")
    be1 = nc.dram_tensor("be1", [128, H1 // 128], F32, kind="ExternalInput")
    bz = nc.dram_tensor("bz", [128, Z // 128], F32, kind="ExternalInput")
    bd0 = nc.dram_tensor("bd0", [128, H1 // 128], F32, kind="ExternalInput")
    bd1 = nc.dram_tensor("bd1", [128, H0 // 128], F32, kind="ExternalInput")
    bd2 = nc.dram_tensor("bd2", [1, F], BF16, kind="ExternalInput")

    z_out = nc.dram_tensor("z_out", [NLOC, Z], F32, kind="ExternalOutput")
    xh_out = nc.dram_tensor("xh_out", [NLOC, F], F32, kind="ExternalOutput")

    with tile.TileContext(nc) as tc:
        with (
            tc.tile_pool(name="const", bufs=1) as const,
            tc.tile_pool(name="acts", bufs=1) as acts,
            tc.tile_pool(name="adj_p", bufs=4) as adj_p,
            tc.tile_pool(name="t_p", bufs=8) as t_p,
            tc.tile_pool(name="io_p", bufs=4) as io_p,
            tc.tile_pool(name="small", bufs=6) as small,
            tc.tile_pool(name="ps", bufs=8, space="PSUM") as ps_p,
            tc.tile_pool(name="dram", bufs=1, space="DRAM") as dram,
        ):
            # ---- replicated constants in SBUF ----
            We0_sb = const.tile([128, KF, H0], BF16)
            nc.sync.dma_start(We0_sb[:], We0.rearrange("(k p) h -> p k h", p=128))
            We1_sb = const.tile([128, H0 // 128, H1], BF16)
            nc.sync.dma_start(We1_sb[:], We1.rearrange("(k p) h -> p k h", p=128))
            Wz_sb = const.tile([128, H1 // 128, Z], BF16)
            nc.sync.dma_start(Wz_sb[:], Wz.rearrange("(k p) h -> p k h", p=128))
            Wd0_sb = const.tile([128, H1], BF16)
            nc.sync.dma_start(Wd0_sb[:], Wd0[:, :])
            Wd1_sb = const.tile([128, H1 // 128, H0], BF16)
            nc.sync.dma_start(Wd1_sb[:], Wd1.rearrange("(k p) h -> p k h", p=128))
            Wd2_sb = const.tile([128, H0 // 128, F], BF16)
            nc.sync.dma_start(Wd2_sb[:], Wd2.rearrange("(k p) h -> p k h", p=128))
            be0_sb = const.tile([128, H0 // 128], F32)
            nc.sync.dma_start(be0_sb[:], be0[:, :])
            be1_sb = const.tile([128, H1 // 128], F32)
            nc.sync.dma_start(be1_sb[:], be1[:, :])
            bz_sb = const.tile([128, Z // 128], F32)
            nc.sync.dma_start(bz_sb[:], bz[:, :])
            bd0_sb = const.tile([128, H1 // 128], F32)
            nc.sync.dma_start(bd0_sb[:], bd0[:, :])
            bd1_sb = const.tile([128, H0 // 128], F32)
            nc.sync.dma_start(bd1_sb[:], bd1[:, :])
            bd2_sb = const.tile([1, F], BF16)
            nc.sync.dma_start(bd2_sb[:], bd2[:, :])
            ones_sb = const.tile([1, 128], BF16)
            nc.any.memset(ones_sb[:], 1.0)
            ident = const.tile([128, 128], F32)
            make_identity(nc, ident[:])

            # ---- persistent activations (bf16, transposed layouts) ----
            hT_sb = acts.tile([128, KF, NLOC], BF16)    # (x*mask).T
            h0T_sb = acts.tile([128, H0 // 128, NLOC], BF16)
            h1T_sb = acts.tile([128, H1 // 128, NLOC], BF16)
            zT_sb = acts.tile([128, NLOC], F32)
            zsT_sb = acts.tile([128, NLOC], BF16)       # normalized z, transposed
            u0T_sb = acts.tile([128, H1 // 128, NLOC], BF16)
            u1T_sb = acts.tile([128, H0 // 128, NLOC], BF16)

            # ---- DRAM bounce buffers for the AllGathers ----
            # each AllGather is split into two half-row gathers so the
            # consumer layer can start on half a while half b is in flight
            HM = MT // 2          # 5 m-tiles per half
            HROWS = HM * 128      # 640 rows per half
            t0loc_a = dram.tile([HROWS, H0], BF16)
            t0loc_b = dram.tile([HROWS, H0], BF16)
            t0full_a = dram.tile([NCORES * HROWS, H0], BF16, addr_space="Shared")
            t0full_b = dram.tile([NCORES * HROWS, H0], BF16, addr_space="Shared")
            t1loc_a = dram.tile([HROWS, H1], BF16)
            t1loc_b = dram.tile([HROWS, H1], BF16)
            t1full_a = dram.tile([NCORES * HROWS, H1], BF16, addr_space="Shared")
            t1full_b = dram.tile([NCORES * HROWS, H1], BF16, addr_space="Shared")
            tzloc_a = dram.tile([HROWS, Z], BF16)
            tzloc_b = dram.tile([HROWS, Z], BF16)
            tzfull_a = dram.tile([NCORES * HROWS, Z], BF16, addr_space="Shared")
            tzfull_b = dram.tile([NCORES * HROWS, Z], BF16, addr_space="Shared")

            rg = [list(range(NCORES))]

            def half_src(kg, full_a, full_b):
                # global k-chunk -> (gathered tensor, row offset): row block
                # r of a gathered half holds rank r's HM local chunks
                r, j = divmod(kg, MT)
                if j < HM:
                    return full_a, r * HROWS + j * 128
                return full_b, r * HROWS + (j - HM) * 128

            # consume all half-a chunks (across ranks) before half-b ones
            KORDER = (
                [r * MT + j for j in range(HM) for r in range(NCORES)]
                + [r * MT + j for j in range(HM, MT) for r in range(NCORES)])

            # ======== phase E: hT = (x*mask).T ; t0 = h @ We0 (local rows) ====
            for kf in range(KF):
                xt_t = io_p.tile([128, NLOC], BF16, tag="xt")
                nc.sync.dma_start(xt_t[:], xT[kf * 128:(kf + 1) * 128, :])
                mt_t = io_p.tile([128, NLOC], BF16, tag="mt")
                nc.sync.dma_start(mt_t[:], mT[kf * 128:(kf + 1) * 128, :])
                nc.vector.tensor_mul(hT_sb[:, kf, :], xt_t[:], mt_t[:])

            for h, loc, full in ((0, t0loc_a, t0full_a),
                                 (1, t0loc_b, t0full_b)):
                for mi in range(HM):
                    m = h * HM + mi
                    ps = ps_p.tile([128, H0], F32, tag="ps")
                    for kf in range(KF):
                        nc.tensor.matmul(
                            ps[:], hT_sb[:, kf, m * 128:(m + 1) * 128],
                            We0_sb[:, kf, :],
                            start=(kf == 0), stop=(kf == KF - 1))
                    stage = io_p.tile([128, H0], BF16, tag="stage")
                    nc.scalar.copy(stage[:], ps[:])
                    nc.sync.dma_start(loc[mi * 128:(mi + 1) * 128, :], stage[:])
                nc.gpsimd.collective_compute(
                    "AllGather", mybir.AluOpType.bypass, replica_groups=rg,
                    ins=[loc[:].opt()], outs=[full[:].opt()])

            # ======== phase L1: h0 = relu(adj @ t0 + be0), transposed out ====
            # free-dim groups keep live PSUM banks <= 8
            for grp in ([(0, 512), (512, 512)], [(1024, 256)]):
                g0 = grp[0][0]
                gw = sum(sz for _, sz in grp)
                pss = {}
                for (off, sz) in grp:
                    for hp in range(H0 // 128):
                        pss[(off, hp)] = ps_p.tile([128, sz], F32, tag="ps", name=f"psb_{off}_{hp}")
                for ki, kg in enumerate(KORDER):
                    a_t = adj_p.tile([128, gw], BF16, tag="adj")
                    nc.sync.dma_start(a_t[:], adjT[kg * 128:(kg + 1) * 128,
                                                   g0:g0 + gw])
                    src, roff = half_src(kg, t0full_a, t0full_b)
                    t_t = t_p.tile([128, H0], BF16, tag="tt")
                    nc.sync.dma_start(t_t[:], src[roff:roff + 128, :])
                    for (off, sz) in grp:
                        for hp in range(H0 // 128):
                            nc.tensor.matmul(
                                pss[(off, hp)][:],
                                t_t[:, hp * 128:(hp + 1) * 128],
                                a_t[:, off - g0:off - g0 + sz],
                                start=(ki == 0), stop=(ki == KADJ - 1))
                for (off, sz) in grp:
                    for hp in range(H0 // 128):
                        nc.scalar.activation(
                            h0T_sb[:, hp, off:off + sz], pss[(off, hp)][:],
                            mybir.ActivationFunctionType.Relu,
                            bias=be0_sb[:, hp:hp + 1])

            # ======== phase P1: t1 = h0 @ We1 (local rows) ========
            for h, loc, full in ((0, t1loc_a, t1full_a),
                                 (1, t1loc_b, t1full_b)):
                for mi in range(HM):
                    m = h * HM + mi
                    ps = ps_p.tile([128, H1], F32, tag="ps")
                    for hk in range(H0 // 128):
                        nc.tensor.matmul(
                            ps[:], h0T_sb[:, hk, m * 128:(m + 1) * 128],
                            We1_sb[:, hk, :],
                            start=(hk == 0), stop=(hk == H0 // 128 - 1))
                    stage = io_p.tile([128, H1], BF16, tag="stage")
                    nc.scalar.copy(stage[:], ps[:])
                    nc.sync.dma_start(loc[mi * 128:(mi + 1) * 128, :], stage[:])
                nc.gpsimd.collective_compute(
                    "AllGather", mybir.AluOpType.bypass, replica_groups=rg,
                    ins=[loc[:].opt()], outs=[full[:].opt()])

            # ======== phase L2: h1 = relu(adj @ t1 + be1) ========
            pss = {}
            for (off, sz) in NLOC_CH:
                for hp in range(H1 // 128):
                    pss[(off, hp)] = ps_p.tile([128, sz], F32, tag="ps", name=f"psb_{off}_{hp}")
            for ki, kg in enumerate(KORDER):
                a_t = adj_p.tile([128, NLOC], BF16, tag="adj")
                nc.sync.dma_start(a_t[:], adjT[kg * 128:(kg + 1) * 128, :])
                src, roff = half_src(kg, t1full_a, t1full_b)
                t_t = t_p.tile([128, H1], BF16, tag="tt")
                nc.sync.dma_start(t_t[:], src[roff:roff + 128, :])
                for (off, sz) in NLOC_CH:
                    for hp in range(H1 // 128):
                        nc.tensor.matmul(
                            pss[(off, hp)][:],
                            t_t[:, hp * 128:(hp + 1) * 128],
                            a_t[:, off:off + sz],
                            start=(ki == 0), stop=(ki == KADJ - 1))
            for (off, sz) in NLOC_CH:
                for hp in range(H1 // 128):
                    nc.scalar.activation(
                        h1T_sb[:, hp, off:off + sz], pss[(off, hp)][:],
                        mybir.ActivationFunctionType.Relu,
                        bias=be1_sb[:, hp:hp + 1])

            # ======== phase P2: tz = h1 @ Wz ========
            for h, loc, full in ((0, tzloc_a, tzfull_a),
                                 (1, tzloc_b, tzfull_b)):
                for mi in range(HM):
                    m = h * HM + mi
                    ps = ps_p.tile([128, Z], F32, tag="ps")
                    for hk in range(H1 // 128):
                        nc.tensor.matmul(
                            ps[:], h1T_sb[:, hk, m * 128:(m + 1) * 128],
                            Wz_sb[:, hk, :],
                            start=(hk == 0), stop=(hk == H1 // 128 - 1))
                    stage = io_p.tile([128, Z], BF16, tag="stage")
                    nc.scalar.copy(stage[:], ps[:])
                    nc.sync.dma_start(loc[mi * 128:(mi + 1) * 128, :], stage[:])
                nc.gpsimd.collective_compute(
                    "AllGather", mybir.AluOpType.bypass, replica_groups=rg,
                    ins=[loc[:].opt()], outs=[full[:].opt()])

            # ======== phase L3: z = adj @ tz + bz (no relu) ========
            pss = {}
            for (off, sz) in NLOC_CH:
                pss[off] = ps_p.tile([128, sz], F32, tag="ps", name=f"psb_{off}")
            for ki, kg in enumerate(KORDER):
                a_t = adj_p.tile([128, NLOC], BF16, tag="adj")
                nc.sync.dma_start(a_t[:], adjT[kg * 128:(kg + 1) * 128, :])
                src, roff = half_src(kg, tzfull_a, tzfull_b)
                t_t = t_p.tile([128, Z], BF16, tag="tt")
                nc.sync.dma_start(t_t[:], src[roff:roff + 128, :])
                for (off, sz) in NLOC_CH:
                    nc.tensor.matmul(
                        pss[off][:], t_t[:], a_t[:, off:off + sz],
                        start=(ki == 0), stop=(ki == KADJ - 1))
            for (off, sz) in NLOC_CH:
                nc.scalar.activation(
                    zT_sb[:, off:off + sz], pss[off][:],
                    mybir.ActivationFunctionType.Identity,
                    bias=bz_sb[:, 0:1])

            # ======== phase Z: normalize rows of z, emit z_out and zsT ======
            for m in range(MT):
                ms = slice(m * 128, (m + 1) * 128)
                pst = ps_p.tile([128, 128], F32, tag="ps")
                nc.tensor.transpose(pst[:], zT_sb[:, ms], ident[:])
                # row sum of squares -> norm -> reciprocal
                sq = small.tile([128, 128], F32, tag="sq")
                nc.scalar.square(sq[:], pst[:])
                ssum = small.tile([128, 1], F32, tag="ssum")
                nc.vector.reduce_sum(ssum[:], sq[:], axis=mybir.AxisListType.X)
                nc.scalar.sqrt(ssum[:], ssum[:])
                nc.vector.tensor_scalar_max(ssum[:], ssum[:], 1e-12)
                rinv = small.tile([128, 1], F32, tag="rinv")
                nc.vector.reciprocal(rinv[:], ssum[:])
                zn = small.tile([128, 128], F32, tag="zn")
                nc.vector.tensor_scalar_mul(zn[:], pst[:], rinv[:])
                nc.sync.dma_start(z_out[ms, :], zn[:])
                # transpose normalized block back for the decoder
                pst2 = ps_p.tile([128, 128], F32, tag="ps")
                nc.tensor.transpose(pst2[:], zn[:], ident[:])
                nc.vector.tensor_copy(zsT_sb[:, ms], pst2[:])

            # ======== phase D: decoder ========
            # u0 = relu(z @ Wd0 + bd0), transposed: u0T = Wd0.T @ zsT
            for hp in range(H1 // 128):
                for (off, sz) in NLOC_CH:
                    ps = ps_p.tile([128, sz], F32, tag="ps")
                    nc.tensor.matmul(
                        ps[:], Wd0_sb[:, hp * 128:(hp + 1) * 128],
                        zsT_sb[:, off:off + sz], start=True, stop=True)
                    nc.scalar.activation(
                        u0T_sb[:, hp, off:off + sz], ps[:],
                        mybir.ActivationFunctionType.Relu,
                        bias=bd0_sb[:, hp:hp + 1])
            # u1 = relu(u0 @ Wd1 + bd1), transposed
            for hp in range(H0 // 128):
                for (off, sz) in NLOC_CH:
                    ps = ps_p.tile([128, sz], F32, tag="ps")
                    for kp in range(H1 // 128):
                        nc.tensor.matmul(
                            ps[:], Wd1_sb[:, kp, hp * 128:(hp + 1) * 128],
                            u0T_sb[:, kp, off:off + sz],
                            start=(kp == 0), stop=(kp == H1 // 128 - 1))
                    nc.scalar.activation(
                        u1T_sb[:, hp, off:off + sz], ps[:],
                        mybir.ActivationFunctionType.Relu,
                        bias=bd1_sb[:, hp:hp + 1])
            # x_hat = (u1 @ Wd2 + bd2) * mask, natural layout
            for m in range(MT):
                ms = slice(m * 128, (m + 1) * 128)
                for (off, sz) in F_CH:
                    ps = ps_p.tile([128, sz], F32, tag="ps")
                    for kp in range(H0 // 128):
                        nc.tensor.matmul(
                            ps[:], u1T_sb[:, kp, ms],
                            Wd2_sb[:, kp, off:off + sz],
                            start=(kp == 0), stop=False)
                    nc.tensor.matmul(
                        ps[:], ones_sb[:, :], bd2_sb[:, off:off + sz],
                        start=False, stop=True)
                    mk_t = io_p.tile([128, 512], F32, tag="mk")
                    nc.sync.dma_start(mk_t[:, :sz], mask[ms, off:off + sz])
                    xh_t = io_p.tile([128, 512], F32, tag="xh")
                    nc.vector.tensor_mul(xh_t[:, :sz], ps[:], mk_t[:, :sz])
                    nc.sync.dma_start(xh_out[ms, off:off + sz], xh_t[:, :sz])

    nc.compile()
    return nc


_PROGRAM_CACHE = {}


def _get_program():
    if "nc" not in _PROGRAM_CACHE:
        _PROGRAM_CACHE["nc"] = build_program()
    return _PROGRAM_CACHE["nc"]


def _bias_cols(b, h):
    # [H] f32 -> [128, H/128] with b_out[p, a] = b[a*128 + p]
    return np.ascontiguousarray(
        np.asarray(b, dtype=np.float32).reshape(h // 128, 128).T)


def shard_inputs(x, adj, mask, We0, be0, We1, be1, Wz, bz,
                 Wd0, bd0, Wd1, bd1, Wd2, bd2):
    x = np.asarray(x, dtype=np.float32)
    adj = np.asarray(adj, dtype=np.float32)
    mask = np.asarray(mask, dtype=np.float32)

    We0p = np.zeros((FPAD, H0), NP_BF16)
    We0p[:F] = np.asarray(We0, np.float32).astype(NP_BF16)
    common = {
        "We0": We0p,
        "We1": np.asarray(We1, np.float32).astype(NP_BF16),
        "Wz": np.asarray(Wz, np.float32).astype(NP_BF16),
        "Wd0": np.asarray(Wd0, np.float32).astype(NP_BF16),
        "Wd1": np.asarray(Wd1, np.float32).astype(NP_BF16),
        "Wd2": np.asarray(Wd2, np.float32).astype(NP_BF16),
        "be0": _bias_cols(be0, H0),
        "be1": _bias_cols(be1, H1),
        "bz": _bias_cols(bz, Z),
        "bd0": _bias_cols(bd0, H1),
        "bd1": _bias_cols(bd1, H0),
        "bd2": np.asarray(bd2, np.float32).astype(NP_BF16).reshape(1, F),
    }

    in_maps = []
    for c in range(NCORES):
        r0 = c * NLOC
        r1 = min((c + 1) * NLOC, N)
        nr = r1 - r0

        xT_c = np.zeros((FPAD, NLOC), NP_BF16)
        xT_c[:F, :nr] = x[r0:r1].T.astype(NP_BF16)
        mT_c = np.zeros((FPAD, NLOC), NP_BF16)
        mT_c[:F, :nr] = mask[r0:r1].T.astype(NP_BF16)
        adjT_c = np.zeros((NPAD, NLOC), NP_BF16)
        adjT_c[:N, :nr] = adj[r0:r1].T.astype(NP_BF16)
        mask_c = np.zeros((NLOC, F), np.float32)
        mask_c[:nr] = mask[r0:r1]

        in_maps.append({
            "xT": xT_c, "mT": mT_c, "adjT": adjT_c, "mask": mask_c,
            **common,
        })
    return in_maps


def run_sharded(in_maps, trace=False, **kwargs):
    nc = _get_program()
    return run_bass_kernel_spmd(nc, in_maps, core_ids=list(range(NCORES)),
                                trace=trace, **kwargs)


def assemble_outputs(results, mask):
    z = np.concatenate([results[c]["z_out"] for c in range(NCORES)],
                       axis=0)[:N]
    xh = np.concatenate([results[c]["xh_out"] for c in range(NCORES)],
                        axis=0)[:N]
    mask = np.asarray(mask, dtype=np.float32)
    return z, xh, mask


def kernel(x, adj, mask, We0, be0, We1, be1, Wz, bz,
           Wd0, bd0, Wd1, bd1, Wd2, bd2):
    in_maps = shard_inputs(x, adj, mask, We0, be0, We1, be1, Wz, bz,
                           Wd0, bd0, Wd1, bd1, Wd2, bd2)
    res = run_sharded(in_maps)
    return assemble_outputs(res.results, mask)
